# revision 33
# baseline (speedup 1.0000x reference)
"""MinGRU cell on 8 TRN2 NeuronCores.

Math (per batch b):
    g = sigmoid(x @ Wg.T + bg)          # [L, D]
    c = tanh(x @ Wh.T + bh)             # [L, D]
    h_t = g_t * h_{t-1} + (1 - g_t) * c_t   (h_0 init = hidden)

Sharding: data-parallel over batch B=8 -> one batch per core, no collectives.

Device layout: "D on partitions, L on free dim".  All matmul operands are
bf16 (same PE rate as fp32r but FWL-eligible weight loads and half the DMA
bytes); PSUM accumulation stays fp32, activations apply the per-partition
bias in fp32 and emit bf16, the DVE scan keeps an fp32 internal state and
emits bf16 h.  Output is written as bf16 [D, L]; the host transposes and
upcasts.

Startup is DMA-bound, and the DMA queues are PACKET-count bound (~60-75
packets/us/queue, one packet per partition line, Sync ring ~74, ACT ring
~59): every 128-partition piece costs ~1.8-2.2us regardless of bytes.  The
first matmul wave needs wg-kd0 + x-kd0 = 256 packets over two queues, so
the first real matmul lands at its ~11us floor; granules stream kd-wise
(x on ACT, wg on Sync, except x kd6-7 which rides Sync behind wg so
slow-ACT-ring runs don't starve the last waves) and the wave pipeline is
matmul-paced from wave 2.
Wh issues immediately behind them, unpaced (rings are FIFO so wg/x keep
priority; DMA issue->completion is 4-8us, so pacing wh behind phase-1
waves ships it too late and stalls the first c-units).  While the first
granules fly, throwaway matmuls on a memset tile ramp the PE out of its
low-power state, and dummy activations preload both ACT tables.  Biases
ship as one packed [128, 24] tensor (a rearranged [D] load would generate
4-byte packets).

Phase-1 PSUM tags are swapped (e0-3 on the "pc" ring) so phase-2 c-unit
e_j recycles the bank freed by sigmoid(e_j), which completes before
phase 1's matmuls end -- the 8 sigmoids at ~820ns each lag the 216ns stop
cadence, so the natural assignment would stall the PE ~2us at the phase
boundary.

The final unit is split into shrinking pieces (320/128/64); the last piece
writes a small contiguous DRAM tensor split across both rings (store cost
is 128 packets per piece, regardless of piece size); the host stitches it
into the output.  kernel() re-runs on an invariant violation (|h| must be
<= max(1, |h0|)) to absorb a rare (~1/20) cold-start device flake that
returns unbounded garbage in some tile.
"""

import numpy as np

import concourse.bacc as bacc
import concourse.tile as tile
import concourse.mybir as mybir
from concourse import bass_utils

B = 8
L = 4096
D = 1024
P = 128
NCH = 512          # token chunk (one fp32 PSUM bank)
KD = D // P        # 8 contraction blocks
NE = D // P        # 8 output-dim blocks
NCHUNK = L // NCH  # 8 token chunks
NKP = KD // 2      # kd pairs (DMA granules)

F32 = mybir.dt.float32
BF16 = mybir.dt.bfloat16
N_WARM = 30        # 128-token PE warmup matmuls (~3.8us at low pstate);
                   # sized so the warmup stream bridges ALL the way to
                   # first-granule arrival (~11.6us).  Ending early is
                   # worse than ending late: an idle PE drops its power
                   # state and wave 0 then runs at ~2x spacing (measured
                   # 427ns vs 216ns steady -- a ~1.7us ramp tax), while a
                   # late warm-end only delays wave 0 by the overshoot.
TAIL = 128         # final contiguous-store piece (last 128 tokens)
# final-unit piece boundaries: sized so the serial (in-order Vector
# queue) stt+scan chain ends as early as possible after the last matmul:
# a small first piece lets the chain start early, a small last piece
# keeps the post-matmul tanh+stt+scan latency short.
TAIL_PIECES = [(0, 160), (160, 384), (384, 512)]


def build_nc():
    nc = bacc.Bacc("TRN2", target_bir_lowering=False, debug=False)

    xq = nc.dram_tensor("xq", [P, NCHUNK, KD, NCH], BF16, kind="ExternalInput").ap()
    wgq = nc.dram_tensor("wgq", [P, KD, D], BF16, kind="ExternalInput").ap()
    whq = nc.dram_tensor("whq", [P, KD, D], BF16, kind="ExternalInput").ap()
    # packed per-partition constants: [bg | bh | h0], each [P, NE]
    bctl = nc.dram_tensor("bctl", [P, 3 * NE], F32, kind="ExternalInput").ap()
    outT = nc.dram_tensor("outT", [D, L], BF16, kind="ExternalOutput").ap()
    out_tail = nc.dram_tensor("out_tail", [P, TAIL], BF16, kind="ExternalOutput").ap()

    out_r = outT.rearrange("(e p) l -> p e l", p=P)     # [128, 8, 4096]

    ACT = mybir.ActivationFunctionType
    ALU = mybir.AluOpType

    with tile.TileContext(nc) as tc:
        with (
            tc.tile_pool(name="const", bufs=1) as const,
            tc.tile_pool(name="xin", bufs=2) as xpool,
            tc.tile_pool(name="gc", bufs=3) as gc,
            tc.tile_pool(name="hout", bufs=2) as hpool,
            tc.tile_pool(name="psum", bufs=4, space="PSUM") as pp,
        ):
            # ---- startup DMAs first, kd-pair granules.
            # x chunk 0 on the ACT ring; wg on the Sync ring.  DMA queues
            # are packet-count bound (~60-75 packets/us, one packet per
            # partition line), so every 128-partition piece costs ~1.8-2.2us
            # regardless of bytes: whole-kd pieces are the right granularity
            # (finer splits double the packet bill for no latency win).
            xin0_p = []
            for k in range(NKP):
                t = xpool.tile([P, 2, NCH], BF16, tag=f"xin0p{k}", name=f"xin0_p{k}")
                if k == 0:
                    # split the lead granule so kd=0 matmuls unblock on the
                    # first half (subtile deps), halving time-to-first-mm
                    nc.scalar.dma_start(out=t[:, 0, :], in_=xq[:, 0, 0, :])
                    nc.scalar.dma_start(out=t[:, 1, :], in_=xq[:, 0, 1, :])
                elif k == NKP - 1:
                    # last x pair rides the (faster) Sync ring behind wg so
                    # slow-ACT-ring runs don't starve waves 6-7; emitted in
                    # the wg loop below to keep Sync queue order wg-first
                    pass
                else:
                    nc.scalar.dma_start(out=t, in_=xq[:, 0, 2 * k : 2 * k + 2, :])
                xin0_p.append(t)

            wg_p = []
            for k in range(NKP):
                t = const.tile([P, 2, D], BF16, name=f"wg_p{k}")
                if k == 0:
                    nc.sync.dma_start(out=t[:, 0, :], in_=wgq[:, 0, :])
                    nc.sync.dma_start(out=t[:, 1, :], in_=wgq[:, 1, :])
                else:
                    nc.sync.dma_start(out=t, in_=wgq[:, 2 * k : 2 * k + 2, :])
                wg_p.append(t)
            nc.sync.dma_start(
                out=xin0_p[NKP - 1], in_=xq[:, 0, 2 * (NKP - 1) :, :]
            )

            def wg_sl(kd, esl):
                return wg_p[kd // 2][:, kd % 2, esl]

            def xin0_sl(kd, t0=0, tn=NCH):
                return xin0_p[kd // 2][:, kd % 2, t0:tn]

            # Wh issues immediately behind wg/x on both rings, UNPACED: the
            # rings process descriptors FIFO so wg/x keep priority, and the
            # issue->completion latency of a 256 KiB piece is 4-8us -- pacing
            # wh behind phase-1 waves ships it far too late (measured: the
            # first c-unit's kd2/kd4 LDWEIGHTS stalled on wh until ~30us).
            # First three pairs ride the Sync ring (drains wg ~17.5us),
            # the last pair the ACT ring (drains x ~19.6us).
            wh_p = []
            for k in range(NKP):
                t = const.tile([P, 2, D], BF16, name=f"wh_p{k}")
                eng = nc.sync if k < 3 else nc.scalar
                eng.dma_start(out=t, in_=whq[:, 2 * k : 2 * k + 2, :])
                wh_p.append(t)

            # ---- PE warmup on a memset tile (gpsimd memsets run right after
            # the preamble) + dummy activations to preload both ACT tables.
            warm_w = const.tile([P, P], BF16, name="warm_w")
            warm_x = const.tile([P, P], BF16, name="warm_x")
            nc.gpsimd.memset(warm_w, 0.0)
            nc.gpsimd.memset(warm_x, 0.0)

            bc_sb = const.tile([P, 3 * NE], F32, name="bc_sb")
            nc.gpsimd.dma_start(out=bc_sb, in_=bctl)
            bg_sb = bc_sb[:, 0:NE]
            bh_sb = bc_sb[:, NE : 2 * NE]
            h0_sb = bc_sb[:, 2 * NE : 3 * NE]

            wps = pp.tile([P, NCH], F32, tag="pg", name="warm_ps")
            last_warm = None
            for i in range(N_WARM):
                last_warm = nc.tensor.matmul(
                    wps[:, 0:P], lhsT=warm_w, rhs=warm_x,
                    start=(i == 0), stop=(i == N_WARM - 1),
                )
            dummy_act = gc.tile([P, 1], BF16, tag="c", name="dummy_act")
            nc.scalar.activation(
                out=dummy_act, in_=warm_x[:, 0:1], func=ACT.Sigmoid, bias=0.0
            )
            nc.scalar.activation(
                out=dummy_act, in_=warm_x[:, 0:1], func=ACT.Tanh, bias=0.0
            )

            # ---- chunk 0, phase 1: one kd-outer pass over ALL 8 e-blocks
            # using all 8 PSUM banks (the extra 4 pg tiles borrow the "pc"
            # slots, idle until phase 2).  Each arriving kd granule unblocks
            # 8 matmuls (1.7us of work), so the PE never starves while the
            # startup stream trickles in at ~1us/kd.
            gt0 = [None] * NE
            kd_mm = {}
            first_real_mm = None
            # e0-3 borrow the "pc" ring, e4-7 the "pg" ring: phase-2 c-unit
            # e_j then recycles the bank freed by sigmoid(e_j), which
            # completes BEFORE phase 1's matmuls end (the 8 sigmoids at
            # ~820ns each lag the 259ns stop cadence, so tying c-unit e0 to
            # sigmoid(e4) -- the old assignment -- would stall the PE ~2us
            # at the phase boundary once wh arrives on time).
            pgs = {
                e: pp.tile(
                    [P, NCH], F32, tag="pc" if e < 4 else "pg",
                    name=f"pg0_{e}",
                )
                for e in range(NE)
            }
            for kd in range(KD):
                for e in range(NE):
                    mm = nc.tensor.matmul(
                        pgs[e],
                        lhsT=wg_sl(kd, slice(e * P, (e + 1) * P)),
                        rhs=xin0_sl(kd),
                        start=(kd == 0),
                        stop=(kd == KD - 1),
                    )
                    if first_real_mm is None:
                        first_real_mm = mm
                        tile.add_dep_helper(
                            mm.ins, last_warm.ins, sync=True,
                            reason="warmup before real mms",
                        )
                kd_mm[kd] = mm

            for e in range(NE):
                g = gc.tile([P, NCH], BF16, tag=f"g{e}", name=f"g0_{e}")
                nc.scalar.activation(
                    out=g, in_=pgs[e], func=ACT.Sigmoid,
                    bias=bg_sb[:, e : e + 1],
                )
                gt0[e] = g

            def wh_sl(kd, esl):
                return wh_p[kd // 2][:, kd % 2, esl]

            prev_h = [None] * NE
            first_c_mm = [None]

            def c_unit(n, e, gtile, xin_sl, t0=0, tn=NCH):
                """c projection + pointwise + scan + store for tokens
                [t0, tn) of chunk n, output block e."""
                w = tn - t0
                lsl = slice(n * NCH + t0, n * NCH + tn)
                esl = slice(e * P, (e + 1) * P)
                pc = pp.tile([P, w], F32, tag="pc", name=f"pc_{n}_{e}_{t0}")
                for kd in range(KD):
                    mm = nc.tensor.matmul(
                        pc,
                        lhsT=wh_sl(kd, esl),
                        rhs=xin_sl(kd, t0, tn),
                        start=(kd == 0),
                        stop=(kd == KD - 1),
                    )
                    if first_c_mm[0] is None:
                        first_c_mm[0] = mm
                c = gc.tile([P, w], BF16, tag="c", name=f"c_{n}_{e}_{t0}")
                nc.scalar.activation(
                    out=c, in_=pc, func=ACT.Tanh, bias=bh_sb[:, e : e + 1]
                )
                d1 = gc.tile([P, w], BF16, tag="d1", name=f"d1_{n}_{e}_{t0}")
                nc.vector.scalar_tensor_tensor(
                    out=d1, in0=gtile[:, t0:tn], scalar=1.0, in1=c,
                    op0=ALU.subtract, op1=ALU.mult,
                )
                if n == 0 and t0 == 0:
                    init = h0_sb[:, e : e + 1]
                else:
                    pw = prev_h[e].shape[-1]
                    init = prev_h[e][:, pw - 1 : pw]
                # tail pieces get dedicated slots: with the shared tag the
                # 2-deep rotation would stall piece q's scan on piece q-2's
                # store ACK (~1.4us) at the very end of the kernel.
                htag = f"htail{t0}" if (n == NCHUNK - 1 and e == NE - 1 and w != NCH) \
                    else f"h{e}"
                h = hpool.tile([P, w], BF16, tag=htag, name=f"h_{n}_{e}_{t0}")
                nc.vector.tensor_tensor_scan(
                    out=h, data0=gtile[:, t0:tn], data1=d1, initial=init,
                    op0=ALU.mult, op1=ALU.subtract,
                )
                prev_h[e] = h
                if n == NCHUNK - 1 and e == NE - 1 and t0 == NCH - TAIL:
                    # final piece: contiguous store, split by partition half
                    # across BOTH rings (DMA cost is per partition line, so
                    # halving the partitions halves the store latency)
                    nc.sync.dma_start(out=out_tail[0 : P // 2, :], in_=h[0 : P // 2, :])
                    nc.scalar.dma_start(out=out_tail[P // 2 :, :], in_=h[P // 2 :, :])
                elif n == NCHUNK - 1 and e == NE - 1 and t0 == 160:
                    nc.scalar.dma_start(out=out_r[:, e, lsl], in_=h)
                else:
                    nc.sync.dma_start(out=out_r[:, e, lsl], in_=h)

            # ---- chunk 0, phase 2
            for e in range(NE):
                c_unit(0, e, gt0[e], xin0_sl)

            # ---- chunks 1..7: interleaved per-e units
            for n in range(1, NCHUNK):
                xin = xpool.tile([P, KD, NCH], BF16, tag="xin", name=f"xin_{n}")
                dma = nc.scalar.dma_start(out=xin, in_=xq[:, n])
                if n == 1:
                    # keep xin1 out of the startup weight stream
                    tile.add_dep_helper(
                        dma.ins, first_c_mm[0].ins, sync=True, reason="pace xin1"
                    )

                def xin_sl(kd, t0, tn, _x=xin):
                    return _x[:, kd, t0:tn]

                for e in range(NE):
                    esl = slice(e * P, (e + 1) * P)
                    pg = pp.tile([P, NCH], F32, tag="pg", name=f"pg_{n}_{e}")
                    for kd in range(KD):
                        nc.tensor.matmul(
                            pg,
                            lhsT=wg_sl(kd, esl),
                            rhs=xin[:, kd, :],
                            start=(kd == 0),
                            stop=(kd == KD - 1),
                        )
                    g = gc.tile([P, NCH], BF16, tag=f"g{e}", name=f"g_{n}_{e}")
                    nc.scalar.activation(
                        out=g, in_=pg, func=ACT.Sigmoid, bias=bg_sb[:, e : e + 1]
                    )
                    if n == NCHUNK - 1 and e == NE - 1:
                        # Final unit: split into pieces so the very last
                        # tanh+stt+scan+store tail is as short as possible
                        # (pieces pipeline across Scalar/Vector while the
                        # PE finishes; sizes balance chain-start vs the
                        # last piece's post-matmul latency).
                        for t0, tn in TAIL_PIECES:
                            c_unit(n, e, g, xin_sl, t0, tn)
                    else:
                        c_unit(n, e, g, xin_sl)

    nc.compile()
    return nc


_NC_CACHE = None


def _get_nc():
    global _NC_CACHE
    if _NC_CACHE is None:
        _NC_CACHE = build_nc()
    return _NC_CACHE


def prep_in_maps(x, hidden, Wg, bg, Wh, bh):
    import ml_dtypes

    bf16 = ml_dtypes.bfloat16
    x = np.asarray(x, dtype=np.float32)
    hidden = np.asarray(hidden, dtype=np.float32)
    bg = np.asarray(bg, dtype=np.float32)
    bh = np.asarray(bh, dtype=np.float32)

    # x [B, L, D] -> xq [B, P, NCHUNK, KD, NCH]
    xbf = x.astype(bf16)
    xq = np.ascontiguousarray(
        xbf.transpose(0, 2, 1)
        .reshape(B, KD, P, NCHUNK, NCH)
        .transpose(0, 2, 3, 1, 4)
    )
    # W [e, d] -> [p, kd, e]
    wgq = np.ascontiguousarray(
        np.asarray(Wg, dtype=np.float32).T.astype(bf16)
        .reshape(KD, P, D).transpose(1, 0, 2)
    )
    whq = np.ascontiguousarray(
        np.asarray(Wh, dtype=np.float32).T.astype(bf16)
        .reshape(KD, P, D).transpose(1, 0, 2)
    )
    # packed constants [P, 3*NE]: columns = [bg | bh | h0] per e-block,
    # feature d = e*P + p  ->  bctl[p, e] = v[e*P + p]
    bctl = np.empty((B, P, 3 * NE), np.float32)
    bctl[:, :, 0:NE] = bg.reshape(NE, P).T[None]
    bctl[:, :, NE : 2 * NE] = bh.reshape(NE, P).T[None]
    bctl[:, :, 2 * NE :] = hidden.reshape(B, NE, P).transpose(0, 2, 1)

    return [
        {
            "xq": xq[b],
            "wgq": wgq,
            "whq": whq,
            "bctl": np.ascontiguousarray(bctl[b]),
        }
        for b in range(B)
    ]


def kernel(x, hidden, Wg, bg, Wh, bh):
    nc = _get_nc()
    in_maps = prep_in_maps(x, hidden, Wg, bg, Wh, bh)
    # MinGRU invariant: h is a convex combination of h0 and tanh outputs,
    # so |h| <= max(1, |h0|).  A rare (~1/20 cold-start) device flake
    # returns unbounded garbage in some tile; detect and re-run.
    bound = max(1.0, float(np.abs(np.asarray(hidden, np.float32)).max())) + 0.5
    for attempt in range(3):
        res = bass_utils.run_bass_kernel_spmd(nc, in_maps, core_ids=list(range(B)))
        outs = []
        for b in range(B):
            oT = np.asarray(res.results[b]["outT"]).copy()    # [D, L] bf16
            tail = np.asarray(res.results[b]["out_tail"])      # [P, TAIL] bf16
            oT[(NE - 1) * P :, L - TAIL :] = tail
            outs.append(oT.T)
        out = np.stack(outs).astype(np.float32)  # [B, L, D]
        if np.isfinite(out).all() and np.abs(out).max() <= bound:
            break
    return np.ascontiguousarray(out)

